# revision 51
# baseline (speedup 1.0000x reference)
"""Trainium2 Bass kernel for nn_MAB_66194035966469 (LIF-gated transformer block).

Data-parallel: batch B=8 maps 1:1 onto 8 NeuronCores; no collectives.

Per-core math (S=1024, D=1024, H=16, hd=64):
  at = lif(X) = (X >= 2);  pre = W @ at + b   (feature-major [D, S])
  qbin/kbin = (pre >= mu + c*sd)   (LN folded into per-(row,col) threshold)
  v' = fc_v(at_k) + bv
  scores_h = (q_h @ k_h.T)/8 * mask ; w = softmax ; out_h = w @ v'_h
  o = out @ Wo.T + bo ; final = out + mish(o)

Attention trick: with m in {0,1} and s >= 0,
  exp(m*s) = 1 + m*(exp(s) - 1) = 1 + w'
so per (head, t-block): ONE ACT exp (PSUM->SBUF bf16) + ONE stt
(w' = (E-1)*m, split DVE/Pool).  The "+1" terms are restored inside the
PV matmul by one extra matmul with stationary vs_aug = sum_tb v_aug and
moving all-ones: it adds sum_t v'[t,d] to every column and 1024 to the
colsum row, so po = unnormalized output and po[64] = exact softmax
denominator.  Normalization is then po / broadcast(colsum) (one Pool
divide into bf16 unnT), with bv exact because sum_t w = 1... (bv is
folded into v' so it normalizes correctly).

Activation tables: phases 1-2 only use {Identity, Copy, Square, Ln, Exp}
(one table); phase 3 adds Mish (one switch).  sd = exp(0.5*ln(var+eps))
avoids the Sqrt table.
"""

import os

import numpy as np
import ml_dtypes

S = 1024          # sequence length (both Sq and Skv)
D = 1024          # feature dim dV
H = 16            # heads
HD = 64           # head dim
NB = D // 128     # 128-partition blocks per feature dim
SH = 512          # s-half (max moving free dim)
NCORES = 8
EPS = 1e-5
POOL_TBS = (2, 5)         # t-blocks whose mask path runs on Pool (fp32)

_CACHE = {}
_LAST = {}


def _patch_tile_drain():
    """This container's walrus rejects instructions carrying more than one
    sem wait.  After Tile finishes scheduling, rewrite every instruction
    with >1 sync waits so the extra waits ride on same-engine NOPs."""
    import concourse.mybir as mybir
    from concourse.tile import TileContext

    if getattr(TileContext, "_mab_wait_split", False):
        return
    orig = TileContext._drain_and_barrier

    def split_sync_waits(nc, max_waits=1):
        for func in nc.m.functions:
            for bb in func.blocks:
                insts = bb.instructions
                if not any(
                    i.sync_info and i.sync_info.on_wait
                    and len(i.sync_info.on_wait) > max_waits
                    for i in insts
                ):
                    continue
                out = []
                for inst in insts:
                    si = inst.sync_info
                    if si and si.on_wait and len(si.on_wait) > max_waits:
                        waits = list(si.on_wait)
                        inst.sync_info = mybir.SyncInfo(
                            on_wait=list(waits[:max_waits]),
                            on_update=list(si.on_update),
                        )
                        for w in waits[max_waits:]:
                            ev = mybir.InstEventSemaphore(
                                name=f"I-waitsplit-{nc.next_id()}",
                                ins=[], outs=[],
                            )
                            ev.engine = inst.engine
                            ev.sync_info = mybir.SyncInfo(
                                on_wait=[w], on_update=[]
                            )
                            out.append(ev)
                    out.append(inst)
                bb.instructions[:] = out

    def _patched(self, tick_clock, wait_clock):
        orig(self, tick_clock, wait_clock)
        split_sync_waits(self.nc)

    TileContext._drain_and_barrier = _patched
    TileContext._mab_wait_split = True


def _build():
    import concourse.bass as bass
    import concourse.mybir as mybir
    from concourse.tile import TileContext

    _patch_tile_drain()

    F32 = mybir.dt.float32
    F32R = mybir.dt.float32r
    BF16 = mybir.dt.bfloat16
    FP8 = mybir.dt.float8e4
    AF = mybir.ActivationFunctionType
    ALU = mybir.AluOpType

    nc = bass.Bass()

    # ---- DRAM I/O (per core) ----
    QT = nc.dram_tensor("QT", [D, S], BF16, kind="ExternalInput")
    KT = nc.dram_tensor("KT", [D, S], BF16, kind="ExternalInput")
    MT = nc.dram_tensor("MT", [S, S], FP8, kind="ExternalInput")   # mask^T
    # fc_q/fc_k weights ship as fp8e4m3: binarization downstream absorbs
    # the quantization (verified < 1e-4 shift in final rel err), and fp8
    # enables DoubleRow matmuls (2 contraction rows / cycle)
    WQT = nc.dram_tensor("WQT", [D, D], FP8, kind="ExternalInput")  # Wq.T
    WKT = nc.dram_tensor("WKT", [D, D], FP8, kind="ExternalInput")
    WVT = nc.dram_tensor("WVT", [D, D], BF16, kind="ExternalInput")
    WOT = nc.dram_tensor("WOT", [D, D], BF16, kind="ExternalInput")  # Wo.T
    C2Q = nc.dram_tensor("C2Q", [2, D], F32R, kind="ExternalInput")  # ones; c
    C2K = nc.dram_tensor("C2K", [2, D], F32R, kind="ExternalInput")
    BQ = nc.dram_tensor("BQ", [1, D], F32, kind="ExternalInput")
    BK = nc.dram_tensor("BK", [1, D], F32, kind="ExternalInput")
    BO = nc.dram_tensor("BO", [1, D], F32, kind="ExternalInput")
    BVREP = nc.dram_tensor("BVREP", [128, D], BF16, kind="ExternalInput")
    ONESB = nc.dram_tensor("ONESB", [128, SH], BF16, kind="ExternalInput")
    ONEC = nc.dram_tensor("ONEC", [128, 64], F32R, kind="ExternalInput")
    OUT = nc.dram_tensor("OUT", [D, S], F32, kind="ExternalOutput")  # final^T

    def bc(t):
        return t.bitcast(F32R)

    with TileContext(nc) as tc:
        with (
            nc.allow_low_precision(reason="bf16/fp32r tiles feed the PE"),
            tc.tile_pool(name="p0", bufs=1) as p0,
            tc.tile_pool(name="pmid", bufs=1) as pmid,
        ):
            # ---------- whole-kernel residents ----------
            ones128 = p0.tile([128, 1], F32R, tag="ones128")
            nc.gpsimd.dma_start(out=ones128[:], in_=ONEC[:, 0:1])
            onesrb = p0.tile([1, 64], F32R, tag="onesrb")
            nc.gpsimd.dma_start(out=onesrb[:], in_=ONEC[0:1, :])
            onesb = p0.tile([128, SH], BF16, tag="onesb")
            nc.gpsimd.dma_start(out=onesb[:], in_=ONESB[:])
            ones128b = onesb[:, 0:1]

            # per-partition bias columns: [128, 4, NB] = (bq, bk, bo, eps)
            colpack = p0.tile([128, 4, NB], F32, tag="colpack")
            eps_t = colpack[:, 3, 0:1]
            nc.vector.memset(eps_t, EPS)
            one_col = colpack[:, 3, 1:2]
            nc.vector.memset(one_col, 1.0)
            bias_cols = {}
            for i, (vn, dram) in enumerate([("BQ", BQ), ("BK", BK),
                                            ("BO", BO)]):
                nc.gpsimd.dma_start(
                    out=colpack[:, i, :],
                    in_=dram[0, :].rearrange("(nb p) -> p nb", p=128),
                )
                bias_cols[vn] = colpack[:, i, :]

            bvrep = p0.tile([128, D], BF16, tag="bvrep")

            # C2 lhsT for the threshold matmul: [2, NB, 128]
            C2 = {}
            for suf, dram in [("q", C2Q), ("k", C2K)]:
                t = p0.tile([2, NB, 128], F32R, tag=f"C2_{suf}")
                nc.gpsimd.dma_start(
                    out=t[:],
                    in_=dram.rearrange("r (nb m) -> r nb m", m=128),
                )
                C2[suf] = t
            MS = {}
            for suf in ["q", "k"]:
                MS[suf] = p0.tile([2, S], F32R, tag=f"MS_{suf}",
                                  name=f"MS_{suf}")

            # stats scratch rows (partition 0)
            st_msq = p0.tile([1, S], F32, tag="st_msq")
            st_mu2 = p0.tile([1, S], F32, tag="st_mu2")
            st_sd = st_mu2  # reused: mu2 consumed before sd written


            # ---------- mid-life big tensors ----------
            mtt = pmid.tile([128, NB, S], FP8, tag="mtt")       # mask^T
            # f32 copies of mask blocks tb in {2,5}: the Pool (GPSIMD) mask
            # path must touch only fp32 (bf16 on Pool is catastrophically
            # slow on hardware)
            mt32 = pmid.tile([128, len(POOL_TBS), S], F32, tag="mt32")
            MTF = nc.dram_tensor("MTF", [len(POOL_TBS) * 128, S], F32,
                                 kind="ExternalInput")
            MTH = nc.dram_tensor("MTH", [128, SH], F32,
                                 kind="ExternalInput")
            mt32h = pmid.tile([128, SH], F32, tag="mt32h")

            def load_masks():
                for tb in range(NB):
                    nc.gpsimd.dma_start(
                        out=mtt[:, tb, :],
                        in_=MT[tb * 128:(tb + 1) * 128, :],
                    )
                for i in range(len(POOL_TBS)):
                    nc.gpsimd.dma_start(
                        out=mt32[:, i, :],
                        in_=MTF[i * 128:(i + 1) * 128, :],
                    )
                nc.gpsimd.dma_start(out=mt32h[:], in_=MTH[:])
            qbinT = pmid.tile([128, NB, S], FP8, tag="qbinT")
            kbinT = pmid.tile([128, NB, S], FP8, tag="kbinT")
            v_aug = pmid.tile([128, NB, H * (HD + 1)], BF16, tag="v_aug")
            vs_aug = pmid.tile([128, H * (HD + 1)], BF16, tag="vs_aug")
            unnT = pmid.tile([128, NB, S], BF16, tag="unnT")
            vview = v_aug[:].rearrange("p tb (h c) -> p tb h c", c=HD + 1)
            nc.vector.memset(vview[:, :, :, HD:HD + 1], 1.0)

            # =========== Phase 1: FC layers + LN thresholds ===========
            with (
                tc.tile_pool(name="p1", bufs=1) as p1,
                tc.tile_pool(name="pw", bufs=2) as pw,
                tc.tile_pool(name="pxs", bufs=3) as pxs,
                tc.tile_pool(name="psq", bufs=16) as psq,
                tc.tile_pool(name="ps1", bufs=4, space="PSUM") as ps1,
                tc.tile_pool(name="ps_st", bufs=2, space="PSUM") as ps_st,
                tc.tile_pool(name="ps_th", bufs=2, space="PSUM") as ps_th,
            ):
                def load_lif(XT, at8, at16=None):
                    for kb in range(NB):
                        xt = pxs.tile([128, S], BF16, tag="xs")
                        nc.sync.dma_start(
                            out=xt[:], in_=XT[kb * 128:(kb + 1) * 128, :]
                        )
                        nc.vector.tensor_scalar(
                            at8[:, kb, :], xt[:], 2.0, None, ALU.is_ge
                        )
                        if at16 is not None:
                            nc.vector.tensor_scalar(
                                at16[:, kb, :], xt[:], 2.0, None, ALU.is_ge
                            )

                def fc_T8(wdram, at8, out_pre, bias_col):
                    """out_pre[n,s] = W @ at + b   (feature-major).

                    fp8 weights x fp8 binary activations in DoubleRow mode:
                    each matmul contracts a 256-row pair of kb-blocks at
                    2 rows/cycle."""
                    for nb in range(NB):
                        wchunk = pw.tile([128, NB, 128], FP8, tag="wt")
                        nc.scalar.dma_start(
                            out=wchunk[:],
                            in_=wdram[:, nb * 128:(nb + 1) * 128].rearrange(
                                "(kb p) m -> p kb m", p=128
                            ),
                        )
                        for sh in range(2):
                            acc = ps1.tile([128, SH], F32, tag="acc")
                            for kk in range(NB // 2):
                                nc.tensor.matmul(
                                    acc[:],
                                    wchunk[:, 2 * kk:2 * kk + 2, :],
                                    at8[:, 2 * kk:2 * kk + 2,
                                        sh * SH:(sh + 1) * SH],
                                    start=(kk == 0), stop=(kk == NB // 2 - 1),
                                    perf_mode=mybir.MatmulPerfMode.DoubleRow,
                                )
                            nc.scalar.activation(
                                out_pre[:, nb, sh * SH:(sh + 1) * SH], acc[:],
                                AF.Identity, bias=bias_col[:, nb:nb + 1],
                                scale=1.0,
                            )

                def ln_stats(pre_t, suf):
                    """MS[suf] rows: mu and sd of pre_t over features."""
                    for sh in range(2):
                        pmu = ps_st.tile([1, SH], F32, tag="pst")
                        for nb in range(NB):
                            nc.tensor.matmul(
                                pmu[:], ones128b,
                                pre_t[:, nb, sh * SH:(sh + 1) * SH],
                                start=(nb == 0), stop=(nb == NB - 1),
                            )
                        nc.scalar.activation(
                            MS[suf][0:1, sh * SH:(sh + 1) * SH], pmu[:],
                            AF.Copy, bias=0.0, scale=1.0 / D,
                        )
                    for sh in range(2):
                        pq = ps_st.tile([1, SH], F32, tag="pst")
                        for nb in range(NB):
                            sq = psq.tile([128, SH], BF16, tag="sq")
                            pslc = pre_t[:, nb, sh * SH:(sh + 1) * SH]
                            nc.scalar.activation(sq[:], pslc, AF.Square)
                            nc.tensor.matmul(
                                pq[:], ones128b, sq[:],
                                start=(nb == 0), stop=(nb == NB - 1),
                            )
                        nc.scalar.activation(
                            st_msq[:, sh * SH:(sh + 1) * SH], pq[:],
                            AF.Copy, bias=0.0, scale=1.0 / D,
                        )
                    mu_row = MS[suf][0:1, :].bitcast(F32)
                    nc.vector.tensor_tensor(st_mu2[:], mu_row, mu_row,
                                            ALU.mult)
                    nc.vector.tensor_tensor(st_msq[:], st_msq[:], st_mu2[:],
                                            ALU.subtract)
                    # sd = exp(0.5*ln(var+eps)) — stays in the exp/ln table
                    nc.scalar.activation(st_sd[:], st_msq[:], AF.Ln,
                                         bias=eps_t[0:1, :], scale=1.0)
                    nc.scalar.activation(st_sd[:], st_sd[:], AF.Exp,
                                         bias=0.0, scale=0.5)
                    nc.sync.dma_start(out=MS[suf][1:2, :],
                                      in_=st_sd[:].bitcast(F32R))

                def lif_norm(pre_t, suf, out_bin):
                    for nb in range(NB):
                        for sh in range(2):
                            th = ps_th.tile([128, SH], F32, tag="th")
                            nc.tensor.matmul(
                                th[:], C2[suf][:, nb, :],
                                MS[suf][:, sh * SH:(sh + 1) * SH],
                                start=True, stop=True,
                            )
                            nc.vector.tensor_tensor(
                                out_bin[:, nb, sh * SH:(sh + 1) * SH],
                                pre_t[:, nb, sh * SH:(sh + 1) * SH],
                                th[:], ALU.is_ge,
                            )


                # --- Q side first (no fc_v dependency) ---
                at8 = p1.tile([128, NB, S], FP8, tag="at_q", name="at_q")
                load_lif(QT, at8)
                pre = p1.tile([128, NB, S], BF16, tag="pre_q", name="pre_q")
                fc_T8(WQT, at8, pre, bias_cols["BQ"])
                nc.gpsimd.dma_start(out=bvrep[:], in_=BVREP[:])
                ln_stats(pre, "q")
                lif_norm(pre, "q", qbinT)

                # --- K side (fp8 for fc_k DoubleRow; bf16 copy feeds the
                # fc_v stationary side) ---
                at8 = p1.tile([128, NB, S], FP8, tag="at_k8", name="at_k8")
                at = at8
                load_lif(KT, at8)
                pre = p1.tile([128, NB, S], BF16, tag="pre_k", name="pre_k")
                fc_T8(WKT, at8, pre, bias_cols["BK"])
                load_masks()
                ln_stats(pre, "k")
                lif_norm(pre, "k", kbinT)

                # fc_v: v' = Wv @ at_k + bv  (natural layout [t, d], bf16)
                for jq in range(2):
                    wv = pw.tile([128, NB, 512], BF16, tag="wtv")
                    nc.scalar.dma_start(
                        out=wv[:],
                        in_=WVT[:, jq * 512:(jq + 1) * 512].rearrange(
                            "(kb p) m -> p kb m", p=128
                        ),
                    )
                    for tb in range(NB):
                        accv = ps1.tile([128, 512], F32, tag="acc")
                        for kb in range(NB):
                            nc.tensor.matmul(
                                accv[:],
                                at[:, kb, tb * 128:(tb + 1) * 128],
                                wv[:, kb, :],
                                start=(kb == 0), stop=(kb == NB - 1),
                            )
                        nc.vector.tensor_tensor(
                            vview[:, tb, jq * 8:(jq + 1) * 8, 0:HD],
                            accv[:].rearrange("p (h c) -> p h c", c=HD),
                            bvrep[:, jq * 512:(jq + 1) * 512].rearrange(
                                "p (h c) -> p h c", c=HD),
                            ALU.add,
                        )

                # vs_aug = sum_tb v_aug (bf16 tree adds on DVE, 2x mode)
                vt0 = p1.tile([128, H * (HD + 1)], BF16, tag="vt0")
                vt1 = p1.tile([128, H * (HD + 1)], BF16, tag="vt1")
                nc.vector.tensor_tensor(vt0[:], v_aug[:, 0, :],
                                        v_aug[:, 1, :], ALU.add)
                nc.vector.tensor_tensor(vt1[:], v_aug[:, 2, :],
                                        v_aug[:, 3, :], ALU.add)
                nc.vector.tensor_tensor(vt0[:], vt0[:], vt1[:], ALU.add)
                nc.vector.tensor_tensor(vt1[:], v_aug[:, 4, :],
                                        v_aug[:, 5, :], ALU.add)
                nc.vector.tensor_tensor(vt0[:], vt0[:], vt1[:], ALU.add)
                nc.vector.tensor_tensor(vt1[:], v_aug[:, 6, :],
                                        v_aug[:, 7, :], ALU.add)
                nc.vector.tensor_tensor(vs_aug[:], vt0[:], vt1[:], ALU.add)


            # =========== Phase 2: attention ===========
            with (
                tc.tile_pool(name="pexp", bufs=6) as pexp,
                tc.tile_pool(name="pwp", bufs=8) as pwp,
                tc.tile_pool(name="pw32", bufs=2) as pw32,
                tc.tile_pool(name="prbs", bufs=3) as prbs,
                tc.tile_pool(name="psr", bufs=3) as psr,
                tc.tile_pool(name="ps_sc", bufs=2, space="PSUM") as ps_sc,
                tc.tile_pool(name="ps_o", bufs=2, space="PSUM") as ps_o,
                tc.tile_pool(name="ps_rb", bufs=2, space="PSUM") as ps_rb,
            ):
                for h in range(H):
                    pp = (h % 2) * 64
                    hb = h // 2
                    hs = slice(h * (HD + 1), (h + 1) * (HD + 1))
                    po = [ps_o.tile([HD + 1, SH], F32, tag="po",
                                    name=f"po_{h}_{i}")
                          for i in range(2)]
                    for tb in range(NB):
                        psc = ps_sc.tile([128, S], F32, tag="psc")
                        for qh in range(2):
                            nc.tensor.matmul(
                                psc[:, qh * SH:(qh + 1) * SH],
                                kbinT[pp:pp + 64, hb,
                                      tb * 128:(tb + 1) * 128],
                                qbinT[pp:pp + 64, hb,
                                      qh * SH:(qh + 1) * SH],
                                start=True, stop=True,
                            )
                        it = h * NB + tb
                        if tb not in POOL_TBS and tb != 7:
                            et = pexp.tile([128, S], BF16, tag="et")
                            nc.scalar.activation(et[:], psc[:], AF.Exp,
                                                 bias=0.0, scale=0.125)
                        if tb in POOL_TBS:
                            # Pool path, all-fp32 inputs: em = E-1;
                            # wp32 = em*m (Pool has no fused stt)
                            et32 = pw32.tile([128, S], F32, tag="et32")
                            nc.scalar.activation(et32[:], psc[:], AF.Exp,
                                                 bias=0.0, scale=0.125)
                            em = pw32.tile([128, S], F32, tag="em32")
                            nc.gpsimd.tensor_scalar_sub(em[:], et32[:], 1.0)
                            wp32 = pw32.tile([128, S], BF16, tag="wp32")
                            nc.gpsimd.tensor_tensor(
                                wp32[:], em[:],
                                mt32[:, POOL_TBS.index(tb), :], ALU.mult)
                            for sh in range(2):
                                nc.tensor.matmul(
                                    po[sh][:],
                                    v_aug[:, tb, hs],
                                    wp32[:, sh * SH:(sh + 1) * SH],
                                    start=(tb == 0), stop=False,
                                )
                        elif tb == 7:
                            # split halves: Pool (fp32 path) takes sh=0,
                            # DVE takes sh=1
                            et32 = pw32.tile([128, S], F32, tag="et32")
                            nc.scalar.activation(et32[:], psc[:], AF.Exp,
                                                 bias=0.0, scale=0.125)
                            em = pw32.tile([128, S], F32, tag="em32")
                            nc.gpsimd.tensor_scalar_sub(
                                em[:, 0:SH], et32[:, 0:SH], 1.0)
                            wp32 = pw32.tile([128, S], BF16, tag="wp32")
                            nc.gpsimd.tensor_tensor(
                                wp32[:, 0:SH], em[:, 0:SH], mt32h[:],
                                ALU.mult)
                            nc.vector.scalar_tensor_tensor(
                                wp32[:, SH:], et32[:, SH:], 1.0,
                                mtt[:, 7, SH:], ALU.subtract, ALU.mult,
                            )
                            for sh in range(2):
                                nc.tensor.matmul(
                                    po[sh][:],
                                    v_aug[:, tb, hs],
                                    wp32[:, sh * SH:(sh + 1) * SH],
                                    start=False, stop=False,
                                )
                        else:
                            wp = pwp.tile([128, S], BF16, tag="wp")
                            nc.vector.scalar_tensor_tensor(
                                wp[:], et[:], 1.0, mtt[:, tb, :],
                                ALU.subtract, ALU.mult,
                            )
                            for sh in range(2):
                                nc.tensor.matmul(
                                    po[sh][:],
                                    v_aug[:, tb, hs],
                                    wp[:, sh * SH:(sh + 1) * SH],
                                    start=(tb == 0), stop=False,
                                )
                    srow = psr.tile([1, S], F32R, tag="sumrow",
                                    name=f"sums_{h}")
                    for sh in range(2):
                        nc.tensor.matmul(
                            po[sh][:], vs_aug[:, hs], onesb[:],
                            start=False, stop=True,
                        )
                        # stage the denominator row Z to SBUF (f32r for the
                        # broadcast matmul); reciprocal comes after the
                        # broadcast so it can use the fast ~51-ULP custom op
                        nc.vector.tensor_copy(
                            srow[:, sh * SH:(sh + 1) * SH],
                            po[sh][HD:HD + 1, :],
                        )
                    for sh in range(2):
                        rb = ps_rb.tile([64, SH], F32, tag="rb")
                        nc.tensor.matmul(
                            rb[:], onesrb[:],
                            srow[:, sh * SH:(sh + 1) * SH],
                            start=True, stop=True,
                        )
                        rbs = prbs.tile([64, SH], F32, tag="rbs")
                        nc.vector.reciprocal_approx_fast(out=rbs[:], in_=rb[:])
                        nc.vector.tensor_tensor(
                            unnT[pp:pp + 64, hb, sh * SH:(sh + 1) * SH],
                            po[sh][0:HD, :], rbs[:], ALU.mult,
                        )

            # =========== Phase 3: fc_o + mish + residual ===========
            with (
                tc.tile_pool(name="pw3", bufs=8) as pw3,
                tc.tile_pool(name="po3", bufs=3) as po3,
                tc.tile_pool(name="pfin", bufs=3) as pfin,
                tc.tile_pool(name="ps3", bufs=4, space="PSUM") as ps3,
            ):
                wos = []
                for nb in range(NB):
                    wo = pw3.tile([128, NB, 128], BF16, tag="wto")
                    nc.scalar.dma_start(
                        out=wo[:],
                        in_=WOT[:, nb * 128:(nb + 1) * 128].rearrange(
                            "(kb p) m -> p kb m", p=128
                        ),
                    )
                    wos.append(wo)
                def mish_front(nb, sh):
                    """fc_o chunk + mish up to t_t; the Pool mult / final
                    add / store run one chunk later (mish_back) so no
                    engine queue stalls on a cross-engine round trip."""
                    wo = wos[nb]
                    acc = ps3.tile([128, SH], F32, tag="acc3")
                    for kb in range(NB):
                        nc.tensor.matmul(
                            acc[:], wo[:, kb, :],
                            unnT[:, kb, sh * SH:(sh + 1) * SH],
                            start=(kb == 0), stop=(kb == NB - 1),
                        )
                    o_t = po3.tile([128, SH], F32, tag="o_t")
                    nc.scalar.activation(
                        o_t[:], acc[:], AF.Identity,
                        bias=bias_cols["BO"][:, nb:nb + 1], scale=1.0,
                    )
                    # mish(o) = o*(w-1)/(w+1), w = (1+e^o)^2
                    # (exp/square live in the same act table as phase 2)
                    e_t = po3.tile([128, SH], F32, tag="e_t")
                    nc.scalar.activation(e_t[:], o_t[:], AF.Exp)
                    w_t = po3.tile([128, SH], F32, tag="w_t")
                    nc.scalar.activation(w_t[:], e_t[:], AF.Square,
                                         bias=one_col[0:128, :],
                                         scale=1.0)
                    d_t = po3.tile([128, SH], F32, tag="d_t")
                    nc.gpsimd.tensor_scalar_add(d_t[:], w_t[:], 1.0)
                    nc.vector.reciprocal_approx_fast(out=d_t[:], in_=d_t[:])
                    t_t = po3.tile([128, SH], F32, tag="t_t")
                    nc.vector.scalar_tensor_tensor(
                        t_t[:], w_t[:], 1.0, d_t[:],
                        ALU.subtract, ALU.mult,
                    )
                    return nb, sh, o_t, t_t

                def mish_back(st):
                    nb, sh, o_t, t_t = st
                    m_t = po3.tile([128, SH], F32, tag="m_t")
                    nc.gpsimd.tensor_tensor(m_t[:], o_t[:], t_t[:],
                                            ALU.mult)
                    f_t = pfin.tile([128, SH], F32, tag="f_t")
                    nc.vector.tensor_tensor(
                        f_t[:],
                        unnT[:, nb, sh * SH:(sh + 1) * SH],
                        m_t[:], ALU.add,
                    )
                    nc.sync.dma_start(
                        out=OUT[nb * 128:(nb + 1) * 128,
                                sh * SH:(sh + 1) * SH],
                        in_=f_t[:],
                    )

                prev3 = None
                for nb in range(NB):
                    for sh in range(2):
                        st = mish_front(nb, sh)
                        if prev3 is not None:
                            mish_back(prev3)
                        prev3 = st
                mish_back(prev3)

    # custom-DVE ops (reciprocal_approx_fast) are extended InstISA
    # subclasses: populate their .instr bytes (Bacc.compile does this;
    # raw Bass + run_bass_kernel_spmd needs the explicit call)
    from concourse.library_overlay import lower_extended_insts
    lower_extended_insts(nc)
    return nc


def kernel(Q, K, adj_mask, Wq, bq, Wk, bk, Wv, bv, Wo, bo,
           g_q, be_q, g_k, be_k):
    from concourse.bass_utils import run_bass_kernel_spmd

    if "nc" not in _CACHE:
        _CACHE["nc"] = _build()
    nc = _CACHE["nc"]

    f32 = np.float32
    bf16 = ml_dtypes.bfloat16
    ones = np.ones
    c_q = ((2.0 - np.asarray(be_q, f32)) / np.asarray(g_q, f32))
    c_k = ((2.0 - np.asarray(be_k, f32)) / np.asarray(g_k, f32))
    f8 = ml_dtypes.float8_e4m3fn
    shared = {
        "WQT": np.ascontiguousarray(Wq.T, dtype=f32).astype(f8),
        "WKT": np.ascontiguousarray(Wk.T, dtype=f32).astype(f8),
        "WVT": np.ascontiguousarray(Wv.T, dtype=f32).astype(bf16),
        "WOT": np.ascontiguousarray(Wo.T, dtype=f32).astype(bf16),
        "C2Q": np.stack([np.ones(D, f32), c_q]).astype(f32),
        "C2K": np.stack([np.ones(D, f32), c_k]).astype(f32),
        "BQ": np.ascontiguousarray(bq, dtype=f32).reshape(1, D),
        "BK": np.ascontiguousarray(bk, dtype=f32).reshape(1, D),
        "BO": np.ascontiguousarray(bo, dtype=f32).reshape(1, D),
        "BVREP": np.ascontiguousarray(
            np.broadcast_to(np.asarray(bv, f32).reshape(1, D),
                            (128, D))).astype(bf16),
        "ONESB": ones((128, SH), f32).astype(bf16),
        "ONEC": ones((128, 64), f32),
    }
    in_maps = []
    for b in range(NCORES):
        m = dict(shared)
        m["QT"] = np.ascontiguousarray(np.asarray(Q[b], dtype=f32).T).astype(bf16)
        m["KT"] = np.ascontiguousarray(np.asarray(K[b], dtype=f32).T).astype(bf16)
        mt_full = np.ascontiguousarray(np.asarray(adj_mask[b, 0],
                                                  dtype=f32).T)
        m["MT"] = mt_full.astype(ml_dtypes.float8_e4m3fn)
        m["MTF"] = np.concatenate([mt_full[tb * 128:(tb + 1) * 128]
                                   for tb in POOL_TBS])
        m["MTH"] = np.ascontiguousarray(mt_full[7 * 128:, 0:SH])
        in_maps.append(m)

    trace = bool(int(os.environ.get("MAB_TRACE", "0")))
    res = run_bass_kernel_spmd(nc, in_maps, list(range(NCORES)), trace=trace)
    _LAST["res"] = res
    _CACHE["in_maps"] = in_maps
    out = np.stack([res.results[b]["OUT"].T for b in range(NCORES)])
    return np.ascontiguousarray(out).astype(np.float32)


def _make_runner(nc, in_maps, n_cores, loop_iters=1):
    """Replicate bass2jax.run_bass_via_pjrt's sharded execution with inputs
    pre-staged on device, so repeated calls measure device execution only."""
    import jax
    import numpy as np
    import concourse.mybir as mybir
    from jax.sharding import Mesh, NamedSharding, PartitionSpec
    from jax.experimental.shard_map import shard_map
    from concourse.bass2jax import (
        _bass_exec_p, install_neuronx_cc_hook, partition_id_tensor,
    )

    install_neuronx_cc_hook()
    pname = nc.partition_id_tensor.name if nc.partition_id_tensor else None
    in_names, out_names, out_avals, zero_outs = [], [], [], []
    for alloc in nc.m.functions[0].allocations:
        if not isinstance(alloc, mybir.MemoryLocationSet):
            continue
        name = alloc.memorylocations[0].name
        if alloc.kind == "ExternalInput":
            if name != pname:
                in_names.append(name)
        elif alloc.kind == "ExternalOutput":
            out_names.append(name)
            shape = tuple(alloc.tensor_shape)
            dtype = mybir.dt.np(alloc.dtype)
            out_avals.append(jax.core.ShapedArray(shape, dtype))
            zero_outs.append(np.zeros(shape, dtype))
    n_params = len(in_names)
    all_names = in_names + out_names
    if pname is not None:
        all_names = all_names + [pname]

    def _body(*args):
        operands = list(args)
        if pname is not None:
            operands.append(partition_id_tensor())
        outs = _bass_exec_p.bind(
            *operands,
            out_avals=tuple(out_avals),
            in_names=tuple(all_names),
            out_names=tuple(out_names),
            lowering_input_output_aliases=(),
            sim_require_finite=True,
            sim_require_nnan=True,
            nc=nc,
        )
        return tuple(outs)

    devices = jax.devices()[:n_cores]
    mesh = Mesh(np.asarray(devices), ("core",))
    spec = PartitionSpec("core")
    sharded = jax.jit(
        shard_map(_body, mesh=mesh,
                  in_specs=(spec,) * (n_params + len(out_names)),
                  out_specs=(spec,) * len(out_names), check_rep=False),
        keep_unused=True,
    )
    concat = [
        np.concatenate([np.asarray(in_maps[c][nm]) for c in range(n_cores)],
                       axis=0)
        for nm in in_names
    ] + [
        np.zeros((n_cores * z.shape[0], *z.shape[1:]), z.dtype)
        for z in zero_outs
    ]
    sh = NamedSharding(mesh, spec)
    dev_args = [jax.device_put(a, sh) for a in concat]

    def run(n=1):
        outs = None
        for _ in range(n):
            outs = sharded(*dev_args)
        jax.block_until_ready(outs)
        return outs

    return run


def bench(iters=32, reps=3):
    """Per-execution device time via async-dispatch pipelining."""
    import time

    assert "nc" in _CACHE and "in_maps" in _CACHE, "run kernel() first"
    run = _make_runner(_CACHE["nc"], _CACHE["in_maps"], NCORES)
    run()

    def timeit(n):
        best = float("inf")
        for _ in range(reps):
            t0 = time.perf_counter()
            run(n)
            best = min(best, time.perf_counter() - t0)
        return best

    # the axon-tunneled fleet is shared and timing is noisy in both
    # directions: repeat the measurement and take the median positive
    # estimate
    ests = []
    for _ in range(5):
        t1 = timeit(1)
        tN = timeit(iters)
        est = (tN - t1) / (iters - 1)
        if est > 0:
            ests.append(est)
    ests.sort()
    per_exec = ests[len(ests) // 2] if ests else float("nan")

    import concourse.bass as bass
    import concourse.mybir as mybir
    if "nc_triv" not in _CACHE:
        nct = bass.Bass()
        xt = nct.dram_tensor("x", [1, 128], mybir.dt.float32,
                             kind="ExternalInput")
        yt = nct.dram_tensor("y", [1, 128], mybir.dt.float32,
                             kind="ExternalOutput")
        from concourse.tile import TileContext
        with TileContext(nct) as tc:
            with tc.tile_pool(name="sb", bufs=1) as sb:
                t = sb.tile([1, 128], mybir.dt.float32, tag="t")
                nct.sync.dma_start(out=t[:], in_=xt[:])
                nct.sync.dma_start(out=yt[:], in_=t[:])
        _CACHE["nc_triv"] = nct
    runt = _make_runner(
        _CACHE["nc_triv"],
        [{"x": np.zeros((1, 128), np.float32)} for _ in range(NCORES)],
        NCORES,
    )
    runt()
    t0 = time.perf_counter()
    runt(1)
    tt1 = time.perf_counter() - t0
    t0 = time.perf_counter()
    runt(iters)
    floor = (time.perf_counter() - t0 - tt1) / (iters - 1)
    est = per_exec - max(floor, 0.0)
    if not (0.0 < est < float("inf")):
        est = per_exec  # contended floor measurement; report uncorrected
    return est, t1

